# revision 85
# baseline (speedup 1.0000x reference)
"""DiT-SiTo block kernel builder for one NeuronCore (2 samples per core).

Index conventions (per sample):
  tokens t in [0,1024); window w in [0,256); slot s4 in {0..3}
  src index s in [0,768): s = 3*w + j (reference order)
  window-chunk layout: w = 128*c + p  (c in {0,1}, p = partition)
  gathered src rows: (p, cc) with cc = 3*c + j  ->  s = 3*(128*c+p) + j
  keep positions r in [0,512): r < 256 -> dst of window w=r; else kept src
  block token chunks: chunk c holds positions r in [128c, 128c+128), p = r%128

Precision plan: attention path (qkv, scores, AV, proj) in fp8-e4m3 with
host-side scaling; MLP in bf16; residual stream x1 in bf16.

Schedule: per-sample index tails are interleaved with that sample's block
frontend (LN1 -> V -> QK -> attention -> proj) so sample 1's index tail
(DVE/DMA-bound) runs under sample 0's matmuls.  The joint LN2+MLP backend
follows, with the recover permutation-matmuls fused behind each sample's
last fc2 tile.
"""

from contextlib import ExitStack

import numpy as np

import concourse.bass as bass
import concourse.mybir as mybir
import concourse.tile as tile
from concourse import library_config
from concourse.masks import make_identity

I16 = mybir.dt.int16
F32 = mybir.dt.float32
BF16 = mybir.dt.bfloat16
FP8 = mybir.dt.float8e4
I32 = mybir.dt.int32
AF = mybir.ActivationFunctionType
ALU = mybir.AluOpType
AX = mybir.AxisListType
DR = mybir.MatmulPerfMode.DoubleRow

B2 = 2
N = 1024
D = 1152
DC = D // 128          # 9
H = 16
DH = 72
NW = 256
NS = 768
T = 512
TC = T // 128          # 4
D4 = 4608
BIG = 1.0e4
RSQ_DH = float(1.0 / np.sqrt(DH))

# fp8 scale plan
S_Y = 4.0              # YT = S_Y * ln1(x)
S_WQK = 512.0          # wqk host scale
S_QK = 16.0            # QKT = S_QK * (q or k)
S_WV = 32.0            # wv host scale
S_V = 32.0             # Vaug = S_V * v
S_ET = 2.0             # ET = S_ET * exp(logit); keeps 2*e^logit < 240 (fp8
                       # max) up to logit ~4.8, far beyond this data's range
S_O = 32.0             # OT = S_O * o
S_WP = 512.0           # wproj host scale
QK_EVAC = S_QK / (S_Y * S_WQK)
V_EVAC = S_V / (S_Y * S_WV)
EXP_SCALE = RSQ_DH / (S_QK * S_QK)
EXP_BIAS = float(np.log(S_ET))
PROJ_EVAC = 1.0 / (S_O * S_WP)


def host_constants():
    w = np.arange(NW)
    winbase = (64 * (w >> 4) + 2 * (w & 15)).astype(np.float32)
    iota256 = np.arange(NW, dtype=np.float32)
    jrow = np.arange(3, dtype=np.float32)
    return winbase, iota256, jrow


def retile_weights(inp, **_ignored):
    """Host-side: fold LN affine into the following matmul, retile weights.

    Attention weights quantized to fp8-e4m3 (scaled); MLP weights bf16.
    All layouts are SBUF-partition-major so each preload is one
    contiguous-per-partition DMA.
    """
    import ml_dtypes
    E4 = ml_dtypes.float8_e4m3
    BF = ml_dtypes.bfloat16
    f32 = np.float32

    def q8(a, scale):
        return np.ascontiguousarray(
            np.clip(np.asarray(a, f32) * scale, -240.0, 240.0).astype(E4))

    def qb(a):
        return np.ascontiguousarray(np.asarray(a, f32).astype(BF))

    g1, b1 = np.asarray(inp["ln1_g"], f32), np.asarray(inp["ln1_b"], f32)
    g2, b2 = np.asarray(inp["ln2_g"], f32), np.asarray(inp["ln2_b"], f32)
    wqkv = np.asarray(inp["w_qkv"], f32); bqkv = np.asarray(inp["b_qkv"], f32)
    wfc1 = np.asarray(inp["w_fc1"], f32); bfc1 = np.asarray(inp["b_fc1"], f32)
    wqkv_f = g1[:, None] * wqkv
    bqkv_f = bqkv + b1 @ wqkv
    wfc1_f = g2[:, None] * wfc1
    bfc1_f = bfc1 + b2 @ wfc1

    # column order: [q heads 0-7 | k heads 0-7 | q heads 8-15 | k heads 8-15]
    perm = np.concatenate([
        np.arange(576), D + np.arange(576),
        576 + np.arange(576), D + 576 + np.arange(576)])
    wqk = wqkv_f[:, perm]                                      # [1152, 2304]
    wqk_t = wqk.reshape(DC, 128, 18, 128).transpose(1, 2, 0, 3)
    wv = wqkv_f[:, 2 * D:]                                     # [1152, 1152]
    wv_t = wv.reshape(DC, 128, D).transpose(1, 0, 2)           # [128, 9, 1152]
    wp = np.asarray(inp["w_proj"], f32)
    wp_t = wp.reshape(DC, 128, D).transpose(1, 0, 2)
    wfc1_t = wfc1_f.reshape(DC, 128, 36, 128).transpose(2, 1, 0, 3)
    wfc2 = np.asarray(inp["w_fc2"], f32)
    wfc2_t = wfc2.reshape(36, 128, D).transpose(1, 0, 2)       # [128, 36, 1152]

    bqk = np.ascontiguousarray(bqkv_f[perm].reshape(18, 128).T)  # [128, 18]
    return dict(
        wqk_t=q8(wqk_t, S_WQK), wv_t=q8(wv_t, S_WV), wp_t=q8(wp_t, S_WP),
        wfc1_t=qb(wfc1_t), wfc2_t=qb(wfc2_t),
        bqk=np.ascontiguousarray(bqk * S_QK).astype(f32),
        bv_row=np.ascontiguousarray(bqkv_f[None, 2 * D:] * S_V).astype(f32),
        bproj_row=np.asarray(inp["b_proj"], f32)[None, :].copy(),
        bfc1=np.ascontiguousarray(bfc1_f.reshape(36, 128).T).astype(f32),
        bfc2_row=np.asarray(inp["b_fc2"], f32)[None, :].copy(),
    )


def build(nc, cfg=None):
    cfg = dict(cfg or {})
    dbg = cfg.get("debug", False)
    stop_after = cfg.get("stop_after", None)   # "index" to skip the block

    x_in = nc.dram_tensor("x", (B2, N, D), F32, kind="ExternalInput")
    noise_in = nc.dram_tensor("noise", (B2, NW, 4), F32, kind="ExternalInput")
    wqk_t = nc.dram_tensor("wqk_t", (128, 18, DC, 128), FP8, kind="ExternalInput")
    wv_t = nc.dram_tensor("wv_t", (128, DC, D), FP8, kind="ExternalInput")
    wp_t = nc.dram_tensor("wp_t", (128, DC, D), FP8, kind="ExternalInput")
    wfc1_t = nc.dram_tensor("wfc1_t", (36, 128, DC, 128), BF16,
                            kind="ExternalInput")
    wfc2_t = nc.dram_tensor("wfc2_t", (128, 36, D), BF16, kind="ExternalInput")
    bqk = nc.dram_tensor("bqk", (128, 18), F32, kind="ExternalInput")
    bv_row = nc.dram_tensor("bv_row", (1, D), F32, kind="ExternalInput")
    bproj_row = nc.dram_tensor("bproj_row", (1, D), F32, kind="ExternalInput")
    bfc1 = nc.dram_tensor("bfc1", (128, 36), F32, kind="ExternalInput")
    bfc2_row = nc.dram_tensor("bfc2_row", (1, D), F32, kind="ExternalInput")

    out = nc.dram_tensor("out", (B2, N, D), F32, kind="ExternalOutput")

    wb_np, iota_np, jrow_np = host_constants()
    winbase = nc.inline_tensor(wb_np, name="winbase")
    iota128 = nc.inline_tensor(np.arange(128, dtype=np.float32), name="iota128")
    iota256 = nc.inline_tensor(iota_np, name="iota256")
    iotaNS = nc.inline_tensor(np.arange(NS, dtype=np.float32), name="iotaNS")
    jrow = nc.inline_tensor(jrow_np, name="jrow")

    okind = "ExternalOutput" if dbg else "Internal"
    xn_d = [nc.dram_tensor(f"xn_d{b}", (N, D), F32, kind=okind) for b in range(B2)]
    g_d = [nc.dram_tensor(f"g_d{b}", (N,), I32, kind=okind) for b in range(B2)]
    keep_d = [nc.dram_tensor(f"keep_d{b}", (T,), I32, kind=okind)
              for b in range(B2)]
    mrow_d = [nc.dram_tensor(f"mrow_d{b}", (NS,), F32, kind="Internal")
              for b in range(B2)]
    sidx_d = [nc.dram_tensor(f"sidx_d{b}", (NS,), I16, kind="Internal")
              for b in range(B2)]
    didx_d = [nc.dram_tensor(f"didx_d{b}", (NW,), I16, kind="Internal")
              for b in range(B2)]
    kidx_d = [nc.dram_tensor(f"kidx_d{b}", (T,), I16, kind="Internal")
              for b in range(B2)]
    g32_d = [nc.dram_tensor(f"g32_d{b}", (N,), F32, kind="Internal")
             for b in range(B2)]
    mk_d = [nc.dram_tensor(f"mk_d{b}", (NS,), F32, kind="Internal")
            for b in range(B2)]
    tok_d = [nc.dram_tensor(f"tok_d{b}", (N,), F32, kind="Internal")
             for b in range(B2)]
    val_d = [nc.dram_tensor(f"val_d{b}", (N,), F32, kind="Internal")
             for b in range(B2)]
    if dbg:
        dbg_sc = nc.dram_tensor("dbg_sc", (B2, 128, 2, 4), F32,
                                kind="ExternalOutput")
        dbg_ms = nc.dram_tensor("dbg_ms", (B2, NS), F32, kind="ExternalOutput")
        dbg_best = nc.dram_tensor("dbg_best", (B2, NS), F32, kind="ExternalOutput")
        dbg_rank = nc.dram_tensor("dbg_rank", (B2, NS), F32, kind="ExternalOutput")

    ctx = ExitStack()
    tc = ctx.enter_context(tile.TileContext(nc))

    consts = ctx.enter_context(tc.tile_pool(name="consts", bufs=1))
    nc.gpsimd.load_library(library_config.mlp)
    ident = consts.tile([128, 128], F32)
    io128 = consts.tile([128, 1], F32)
    nc.sync.dma_start(io128[:], bass.AP(
        tensor=iota128, offset=0, ap=[[1, 128], [1, 1]]))
    make_identity(nc, ident)
    identb = consts.tile([128, 128], BF16)
    nc.vector.tensor_copy(identb[:], ident[:])

    # ---- block-persistent state (created before the index pools so those
    # can close underneath it) ----
    bcp = ctx.enter_context(tc.tile_pool(name="bcp", bufs=1))
    bsm = ctx.enter_context(tc.tile_pool(name="bsmall", bufs=4))
    p_x1 = ctx.enter_context(tc.tile_pool(name="p_x1", bufs=1))
    x1 = p_x1.tile([128, B2, TC, D], BF16)

    bqkt = bcp.tile([128, 18], F32)
    nc.sync.dma_start(bqkt[:], bqk[:, :])
    bf1t = bcp.tile([128, 36], F32)
    nc.sync.dma_start(bf1t[:], bfc1[:, :])
    ones72 = bcp.tile([1, DH], BF16)
    nc.vector.memset(ones72[:], 1.0)
    expb = bcp.tile([128, 1], F32)
    nc.vector.memset(expb[:], EXP_BIAS)
    bpb = bcp.tile([128, D], BF16)
    nc.gpsimd.dma_start(bpb[:], bass.AP(
        tensor=bproj_row, offset=0, ap=[[0, 128], [1, D]]))
    bvb = bcp.tile([128, D], BF16)
    nc.gpsimd.dma_start(bvb[:], bass.AP(
        tensor=bv_row, offset=0, ap=[[0, 128], [1, D]]))
    kidx_t = [bcp.tile([128, 32], I16, tag=f"kidx2_{b}", name=f"kidx2_{b}")
              for b in range(B2)]

    # ---- index pools: tail pools first (outlive the head pools) ----
    tpools = ExitStack()
    small = tpools.enter_context(tc.tile_pool(name="small", bufs=4))
    twide = tpools.enter_context(tc.tile_pool(name="twide", bufs=1))
    rows = tpools.enter_context(tc.tile_pool(name="rows", bufs=1))
    xrows = tpools.enter_context(tc.tile_pool(name="xrows", bufs=2))
    tconsts = tpools.enter_context(tc.tile_pool(name="tconsts", bufs=1))

    ioNSb = tconsts.tile([128, NS], F32)
    nc.sync.dma_start(ioNSb[:], bass.AP(
        tensor=iotaNS, offset=0, ap=[[0, 128], [1, NS]]))
    witer = tconsts.tile([1, NW], F32)
    nc.sync.dma_start(witer[:], iota256[None, :])

    hpools = ExitStack()
    hconsts = hpools.enter_context(tc.tile_pool(name="hconsts", bufs=1))
    hwide = hpools.enter_context(tc.tile_pool(name="hwide", bufs=1))
    hrows = hpools.enter_context(tc.tile_pool(name="hrows", bufs=1))
    idxp = hpools.enter_context(tc.tile_pool(name="idxp", bufs=2))
    scw = hpools.enter_context(tc.tile_pool(name="scw", bufs=1))
    simp = hpools.enter_context(tc.tile_pool(name="simp", bufs=1))
    xsp = hpools.enter_context(tc.tile_pool(name="xsp", bufs=1))
    psumS = hpools.enter_context(tc.tile_pool(name="psumS", bufs=3, space="PSUM"))
    psumT = hpools.enter_context(tc.tile_pool(name="psumT", bufs=3, space="PSUM"))

    iotaBIG = hconsts.tile([128, NW], F32)
    nc.sync.dma_start(iotaBIG[:], bass.AP(
        tensor=iota256, offset=0, ap=[[0, 128], [1, NW]]))
    nc.vector.tensor_scalar_add(iotaBIG[:], iotaBIG[:], BIG)
    jb = hconsts.tile([128, 3], F32)
    nc.sync.dma_start(jb[:], bass.AP(tensor=jrow, offset=0, ap=[[0, 128], [1, 3]]))
    wbt = hconsts.tile([128, 2], F32)
    for c in range(2):
        nc.sync.dma_start(
            wbt[:, c:c + 1],
            bass.AP(tensor=winbase, offset=128 * c, ap=[[1, 128], [1, 1]]))

    # =================== index pipeline: heads + sims ===================
    def slot_x_ap(b, s4, c):
        sy, sx = s4 >> 1, s4 & 1
        return bass.AP(
            tensor=x_in, offset=(b * N + 512 * c + 32 * sy + sx) * D,
            ap=[[64 * D, 8], [2 * D, 16], [1, D]])

    def slot_xn_ap(b, s4, c):
        sy, sx = s4 >> 1, s4 & 1
        return bass.AP(
            tensor=xn_d[b], offset=(512 * c + 32 * sy + sx) * D,
            ap=[[64 * D, 8], [2 * D, 16], [1, D]])

    xstate = [None, None]
    for b in range(B2):
        if True:
            xs = xsp.tile([128, 4, 2, D], F32, tag="xs")
            ldq = nc.sync if b == 0 else nc.scalar
            for s4 in range(4):
                for c in range(2):
                    ldq.dma_start(xs[:, s4, c, :], slot_x_ap(b, s4, c))
            ss8 = small.tile([128, 8], F32, tag="ss8")
            for s4 in range(4):
                for c in range(2):
                    sq = idxp.tile([128, D], F32, tag="scr")
                    nc.vector.scalar_tensor_tensor(
                        sq[:], xs[:, s4, c, :], 1.0, xs[:, s4, c, :],
                        op0=ALU.bypass, op1=ALU.mult,
                        accum_out=ss8[:, 2 * s4 + c:2 * s4 + c + 1])
            s08 = small.tile([128, 8], F32, tag="s08")
            nc.scalar.activation(s08[:], ss8[:], AF.Sqrt)
            rs8 = small.tile([128, 8], F32, tag="rs8")
            t8 = small.tile([128, 8], F32, tag="t8")
            nc.vector.reciprocal(rs8[:], s08[:])
            for _ in range(2):
                nc.vector.scalar_tensor_tensor(
                    t8[:], s08[:], -1.0, rs8[:], op0=ALU.mult, op1=ALU.mult)
                nc.vector.tensor_scalar_add(t8[:], t8[:], 2.0)
                nc.vector.tensor_mul(rs8[:], rs8[:], t8[:])
            q8t = small.tile([128, 8], F32, tag="q8")
            nc.vector.tensor_mul(q8t[:], ss8[:], rs8[:])
            nc.vector.tensor_add(q8t[:], q8t[:], s08[:])
            nc.vector.tensor_scalar(
                q8t[:], q8t[:], 0.5, 1e-6, op0=ALU.mult, op1=ALU.add)
            inv8 = small.tile([128, 8], F32, tag="inv8")
            nc.vector.reciprocal(inv8[:], q8t[:])
            for _ in range(2):
                nc.vector.scalar_tensor_tensor(
                    t8[:], q8t[:], -1.0, inv8[:], op0=ALU.mult, op1=ALU.mult)
                nc.vector.tensor_scalar_add(t8[:], t8[:], 2.0)
                nc.vector.tensor_mul(inv8[:], inv8[:], t8[:])
            for s4 in range(4):
                for c in range(2):
                    nc.scalar.activation(
                        xs[:, s4, c, :], xs[:, s4, c, :], AF.Copy,
                        scale=inv8[:, 2 * s4 + c:2 * s4 + c + 1])
                    nc.sync.dma_start(slot_xn_ap(b, s4, c), xs[:, s4, c, :])

            if True:
                W = scw.tile([128, 2, D], F32, tag="W")
                nc.vector.tensor_add(W[:], xs[:, 0, :, :], xs[:, 1, :, :])
                nc.vector.tensor_add(W[:], W[:], xs[:, 2, :, :])
                nc.vector.tensor_add(W[:], W[:], xs[:, 3, :, :])
                dots = small.tile([128, 2, 4], F32, tag="dots")
                for s4 in range(4):
                    for c in range(2):
                        scr = idxp.tile([128, D], F32, tag="scr")
                        nc.vector.scalar_tensor_tensor(
                            scr[:], xs[:, s4, c, :], 1.0, W[:, c, :],
                            op0=ALU.bypass, op1=ALU.mult,
                            accum_out=dots[:, c, s4:s4 + 1])
                nt = small.tile([128, 2, 4], F32, tag="nt")
                for c in range(2):
                    nc.sync.dma_start(
                        nt[:, c, :],
                        bass.AP(tensor=noise_in, offset=(b * NW + 128 * c) * 4,
                                ap=[[4, 128], [1, 4]]))
                sc = small.tile([128, 2, 4], F32, tag="scsc")
                nc.vector.tensor_scalar_mul(nt[:], nt[:], 0.1)
                nc.vector.scalar_tensor_tensor(
                    sc[:], dots[:], 0.25, nt[:], op0=ALU.mult, op1=ALU.add)
                if dbg:
                    nc.sync.dma_start(dbg_sc[b], sc[:])

                # argmax over the 4 slots, first max wins:
                # dl = (1-e0) * (1 + (1-e1) * (2 - e2))
                mm = small.tile([128, 2], F32, tag="mm")
                m23 = small.tile([128, 2], F32, tag="m23")
                nc.vector.tensor_tensor(mm[:], sc[:, :, 0], sc[:, :, 1], op=ALU.max)
                nc.vector.tensor_tensor(m23[:], sc[:, :, 2], sc[:, :, 3], op=ALU.max)
                nc.vector.tensor_tensor(mm[:], mm[:], m23[:], op=ALU.max)
                e0 = small.tile([128, 2], F32, tag="e0")
                e1 = small.tile([128, 2], F32, tag="e1")
                e2 = small.tile([128, 2], F32, tag="e2")
                nc.vector.tensor_tensor(e0[:], sc[:, :, 0], mm[:], op=ALU.is_equal)
                nc.vector.tensor_tensor(e1[:], sc[:, :, 1], mm[:], op=ALU.is_equal)
                nc.vector.tensor_tensor(e2[:], sc[:, :, 2], mm[:], op=ALU.is_equal)
                u2 = small.tile([128, 2], F32, tag="u2")
                nc.vector.tensor_scalar(
                    u2[:], e2[:], -1.0, 2.0, op0=ALU.mult, op1=ALU.add)
                v1 = small.tile([128, 2], F32, tag="v1")
                nc.vector.scalar_tensor_tensor(
                    v1[:], e1[:], -1.0, u2[:], op0=ALU.mult, op1=ALU.mult)
                u1 = small.tile([128, 2], F32, tag="u1")
                nc.vector.tensor_add(u1[:], v1[:], u2[:])
                nc.vector.tensor_scalar_add(u1[:], u1[:], 1.0)
                v0 = small.tile([128, 2], F32, tag="v0")
                nc.vector.scalar_tensor_tensor(
                    v0[:], e0[:], -1.0, u1[:], op0=ALU.mult, op1=ALU.mult)
                dl = small.tile([128, 2], F32, tag="dl")
                nc.vector.tensor_add(dl[:], v0[:], u1[:])

                # dst token = winbase + 32*(dl>>1) + (dl&1)
                syt = small.tile([128, 2], F32, tag="syt")
                nc.vector.tensor_scalar(syt[:], dl[:], 2.0, None, op0=ALU.is_ge)
                sxt = small.tile([128, 2], F32, tag="sxt")
                nc.vector.scalar_tensor_tensor(
                    sxt[:], syt[:], -2.0, dl[:], op0=ALU.mult, op1=ALU.add)
                dt = small.tile([128, 2], F32, tag="dt")
                nc.vector.scalar_tensor_tensor(
                    dt[:], syt[:], 32.0, sxt[:], op0=ALU.mult, op1=ALU.add)
                nc.vector.tensor_add(dt[:], dt[:], wbt[:])

                # src tokens [128, 2, 3]
                st = small.tile([128, 2, 3], F32, tag="st")
                for c in range(2):
                    ge = small.tile([128, 3], F32, tag="ge")
                    nc.vector.tensor_scalar(
                        ge[:], jb[:], dl[:, c:c + 1], None, op0=ALU.is_ge)
                    sl = small.tile([128, 3], F32, tag="sl")
                    nc.vector.tensor_add(sl[:], ge[:], jb[:])
                    sy2 = small.tile([128, 3], F32, tag="sy2")
                    nc.vector.tensor_scalar(
                        sy2[:], sl[:], 2.0, None, op0=ALU.is_ge)
                    sx2 = small.tile([128, 3], F32, tag="sx2")
                    nc.vector.scalar_tensor_tensor(
                        sx2[:], sy2[:], -2.0, sl[:], op0=ALU.mult, op1=ALU.add)
                    nc.vector.scalar_tensor_tensor(
                        st[:, c, :], sy2[:], 32.0, sx2[:],
                        op0=ALU.mult, op1=ALU.add)
                    nc.vector.tensor_scalar_add(
                        st[:, c, :], st[:, c, :], wbt[:, c:c + 1])

            # ---- rows ----
            srow_s = xrows.tile([1, NS], F32, tag="srow_s")   # s order
            for c in range(2):
                nc.sync.dma_start(
                    srow_s[:, 384 * c:384 * (c + 1)].rearrange(
                        "a (p j) -> a p j", p=128),
                    st[:, c, :])
            drow_w = xrows.tile([1, NW], F32, tag="drow_w")
            for c in range(2):
                nc.sync.dma_start(
                    drow_w[:, 128 * c:128 * (c + 1)], dt[:, c:c + 1])
            # i-order idx rows for dma_gather (i = 128*chunk + p)
            sgrow = hrows.tile([1, NS], F32, tag="sgrow")
            for h2 in range(2):
                for j3 in range(3):
                    nc.sync.dma_start(
                        sgrow[:, 384 * h2 + 128 * j3:384 * h2 + 128 * (j3 + 1)]
                        .rearrange("a (k o) -> a k o", o=1),
                        st[:, h2, j3:j3 + 1])
            sgi = hrows.tile([1, NS], I16, tag="sgi")
            nc.vector.tensor_copy(sgi[:], sgrow[:])
            nc.sync.dma_start(sidx_d[b][None, :], sgi[:])
            sidx_w = hrows.tile([128, 48], I16, tag="sidx_w")
            for g8 in range(8):
                nc.sync.dma_start(sidx_w[16 * g8:16 * (g8 + 1), :], bass.AP(
                    tensor=sidx_d[b], offset=0, ap=[[1, 16], [16, 48]]))
            dgi = hrows.tile([1, NW], I16, tag="dgi")
            nc.vector.tensor_copy(dgi[:], drow_w[:])
            nc.sync.dma_start(didx_d[b][None, :], dgi[:])
            didx_w = hrows.tile([128, 16], I16, tag="didx_w")
            for g8 in range(8):
                nc.sync.dma_start(didx_w[16 * g8:16 * (g8 + 1), :], bass.AP(
                    tensor=didx_d[b], offset=0, ap=[[1, 16], [16, 16]]))

            # ---- C: gather xn rows, transpose, sim (two halves) ----
            msc = small.tile([128, 6], F32, tag="msc")
            bst = small.tile([128, 6], F32, tag="bst")
            if True:
                xnd = simp.tile([128, 2, D], F32, tag="xnd")
                nc.gpsimd.dma_gather(
                    out_ap=xnd[:], in_ap=xn_d[b][:], idxs_ap=didx_w[:],
                    num_idxs=NW, num_idxs_reg=NW, elem_size=D)
                xndT = simp.tile([128, DC, NW], F32, tag="xndT")
                for c in range(2):
                    for dc in range(DC):
                        pt = psumT.tile([128, 128], F32, tag="pt")
                        nc.tensor.transpose(
                            pt[:], xnd[:, c, 128 * dc:128 * (dc + 1)], ident[:])
                        nc.scalar.copy(xndT[:, dc, 128 * c:128 * (c + 1)], pt[:])
                for half in range(2):
                    xns = simp.tile([128, 3, D], F32, tag="xns")
                    nc.gpsimd.dma_gather(
                        out_ap=xns[:], in_ap=xn_d[b][:],
                        idxs_ap=sidx_w[:, 24 * half:24 * (half + 1)],
                        num_idxs=384, num_idxs_reg=384, elem_size=D)
                    xnsT = simp.tile([128, 3, DC, 128], F32, tag="xnsT")
                    for c3 in range(3):
                        for dc in range(DC):
                            pt = psumT.tile([128, 128], F32, tag="pt")
                            nc.tensor.transpose(
                                pt[:], xns[:, c3, 128 * dc:128 * (dc + 1)],
                                ident[:])
                            nc.scalar.copy(xnsT[:, c3, dc, :], pt[:])
                    for c3 in range(3):
                        cc6 = 3 * half + c3
                        ps = psumS.tile([128, NW], F32, tag="ps")
                        for dc in range(DC):
                            nc.tensor.matmul(
                                ps[:], xnsT[:, c3, dc, :], xndT[:, dc, :],
                                start=(dc == 0), stop=(dc == DC - 1))
                        nc.vector.reduce_max(
                            msc[:, cc6:cc6 + 1], ps[:], axis=AX.X)
                        eqt = hwide.tile([128, NW], F32, tag="eqt")
                        nc.vector.tensor_scalar(
                            eqt[:], ps[:], msc[:, cc6:cc6 + 1], None,
                            op0=ALU.is_equal)
                        mskt = hwide.tile([128, NW], F32, tag="mskt")
                        nc.vector.scalar_tensor_tensor(
                            mskt[:], eqt[:], -BIG, iotaBIG[:],
                            op0=ALU.mult, op1=ALU.add)
                        nc.vector.tensor_reduce(
                            bst[:, cc6:cc6 + 1], mskt[:], axis=AX.X, op=ALU.min)
            xstate[b] = dict(msc=msc, bst=bst, srow_s=srow_s, drow_w=drow_w)
    hpools.close()

    # ---- attention weights, fully resident in fp8 until proj(b1) ----
    wA_scope = ExitStack()
    p_wA = wA_scope.enter_context(tc.tile_pool(name="p_wA", bufs=1))
    wqkA = p_wA.tile([128, 18, DC, 128], FP8, name="wqkA")
    nc.gpsimd.dma_start(wqkA[:], wqk_t[:, :, :, :])
    wvA = p_wA.tile([128, DC, D], FP8, name="wvA")
    nc.gpsimd.dma_start(wvA[:], wv_t[:, :, :])
    wpA = p_wA.tile([128, DC, D], FP8, name="wpA")
    nc.gpsimd.dma_start(wpA[:], wp_t[:, :, :])

    fenv = dict(identb=identb, io128=io128, x_in=x_in,
                wqkA=wqkA, wvA=wvA, wpA=wpA,
                bqkt=bqkt, bvb=bvb, bpb=bpb, expb=expb, ones72=ones72,
                kidx_t=kidx_t, x1=x1, bsm=bsm)

    # ========== per-sample index tails, emitted as generators so sample
    # 1's tail (a DVE/DMA latency chain) can be co-emitted part-by-part
    # between sample 0's frontend stages: the in-order engine queues then
    # match data-readiness order instead of head-of-line blocking ==========
    def index_tail_gen(b):
        if True:
            # sync ring: the frontend keeps the scalar (Act HWDGE) ring busy
            # with LN1/evac/exp activations, so tails ride sync
            ldq = nc.sync
            msc, bst = xstate[b]["msc"], xstate[b]["bst"]
            srow_s, drow_w = xstate[b]["srow_s"], xstate[b]["drow_w"]
            rnk = small.tile([128, 6], F32, tag="rnk")

            # maxsim broadcast via DRAM bounce (s order)
            for cc6 in range(6):
                c, j = cc6 // 3, cc6 % 3
                ldq.dma_start(
                    bass.AP(tensor=mrow_d[b], offset=384 * c + j,
                            ap=[[3, 128], [1, 1]]),
                    msc[:, cc6:cc6 + 1])
            mbc = twide.tile([128, NS], F32, tag="mbc")
            ldq.dma_start(
                mbc[:],
                bass.AP(tensor=mrow_d[b], offset=0, ap=[[0, 128], [1, NS]]))

            # exact stable rank
            gcnt = small.tile([128, 1], F32, tag="gcnt")
            ecnt = small.tile([128, 1], F32, tag="ecnt")
            for lh in range(2):
                for c3 in range(3):
                    cc6 = 3 * lh + c3
                    # ltm[p, jj] = (jj < 3*(128*lh + p) + c3), on the fly
                    ltm = twide.tile([128, NS], F32, tag="ltm")
                    scol = small.tile([128, 1], F32, tag="scol")
                    nc.vector.tensor_scalar(
                        scol[:], io128[:], 3.0, float(384 * lh + c3),
                        op0=ALU.mult, op1=ALU.add)
                    nc.vector.tensor_scalar(
                        ltm[:], ioNSb[:], scol[:], None, op0=ALU.is_lt)
                    sc1 = twide.tile([128, NS], F32, tag="sc12")
                    nc.vector.scalar_tensor_tensor(
                        sc1[:], mbc[:], msc[:, cc6:cc6 + 1], mbc[:],
                        op0=ALU.is_gt, op1=ALU.bypass, accum_out=gcnt[:])
                    sc2 = twide.tile([128, NS], F32, tag="sc12")
                    nc.vector.scalar_tensor_tensor(
                        sc2[:], mbc[:], msc[:, cc6:cc6 + 1], ltm[:],
                        op0=ALU.is_equal, op1=ALU.mult, accum_out=ecnt[:])
                    nc.vector.tensor_add(rnk[:, cc6:cc6 + 1], gcnt[:], ecnt[:])
            yield

            # best + rank rows in s order
            brow = rows.tile([1, NS], F32, tag="brow")
            rrow = rows.tile([1, NS], F32, tag="rrow")
            for cc6 in range(6):
                c, j = cc6 // 3, cc6 % 3
                dst_b = brow[:].rearrange(
                    "a (c p j) -> a p c j", c=2, p=128)[:, :, c, j]
                ldq.dma_start(dst_b, bst[:, cc6:cc6 + 1])
                dst_r = rrow[:].rearrange(
                    "a (c p j) -> a p c j", c=2, p=128)[:, :, c, j]
                ldq.dma_start(dst_r, rnk[:, cc6:cc6 + 1])
            if dbg:
                nc.sync.dma_start(dbg_ms[b][None, :], mbc[0:1, :])
                nc.sync.dma_start(dbg_best[b][None, :], brow[:])
                nc.sync.dma_start(dbg_rank[b][None, :], rrow[:])

            # masks + prefix sum (s order)
            kpm = rows.tile([1, NS], F32, tag="kpm")
            nc.vector.tensor_scalar(kpm[:], rrow[:], 512.0, None, op0=ALU.is_ge)
            kex = rows.tile([1, NS], F32, tag="kex")
            nc.vector.tensor_tensor_scan(
                kex[:], kpm[:], kpm[:], 0.0, op0=ALU.add, op1=ALU.bypass)
            nc.vector.tensor_sub(kex[:], kex[:], kpm[:])
            # v_src = best + kpm*(256 + kex - best)
            tq = rows.tile([1, NS], F32, tag="tmp768")
            nc.vector.tensor_sub(tq[:], kex[:], brow[:])
            nc.vector.scalar_tensor_tensor(
                tq[:], tq[:], 256.0, kpm[:], op0=ALU.add, op1=ALU.mult)
            vsr = rows.tile([1, NS], F32, tag="vsr")
            nc.vector.tensor_add(vsr[:], tq[:], brow[:])
            yield

            # ---- one-hot compaction + g construction (no scatters) ----
            # token/value rows written straight to DRAM: [dst_w | src_s],
            # [witer | vsr]
            ldq.dma_start(
                bass.AP(tensor=tok_d[b], offset=0, ap=[[1, 1], [1, NW]]),
                drow_w[:])
            ldq.dma_start(
                bass.AP(tensor=tok_d[b], offset=NW, ap=[[1, 1], [1, NS]]),
                srow_s[:])
            ldq.dma_start(
                bass.AP(tensor=val_d[b], offset=0, ap=[[1, 1], [1, NW]]),
                witer[:])
            ldq.dma_start(
                bass.AP(tensor=val_d[b], offset=NW, ap=[[1, 1], [1, NS]]),
                vsr[:])
            # masked keep-rank row: kpm*(kex+1) - 1  (pruned -> -1)
            mk = rows.tile([1, NS], F32, tag="mk")
            nc.vector.scalar_tensor_tensor(
                mk[:], kex[:], 1.0, kpm[:], op0=ALU.add, op1=ALU.mult)
            nc.vector.tensor_scalar_add(mk[:], mk[:], -1.0)
            ldq.dma_start(mk_d[b][None, :], mk[:])
            mkb = twide.tile([128, NS], F32, tag="mkb")
            ldq.dma_start(mkb[:], bass.AP(
                tensor=mk_d[b], offset=0, ap=[[0, 128], [1, NS]]))
            tkb = twide.tile([128, N], F32, tag="tkb")
            ldq.dma_start(tkb[:], bass.AP(
                tensor=tok_d[b], offset=0, ap=[[0, 128], [1, N]]))
            stb = tkb[:, NW:]
            vlb = twide.tile([128, N], F32, tag="vlb")
            ldq.dma_start(vlb[:], bass.AP(
                tensor=val_d[b], offset=0, ap=[[0, 128], [1, N]]))
            yield

            krow = rows.tile([1, T], F32, tag="krow")
            nc.vector.tensor_copy(krow[:, :NW], drow_w[:])
            for c2 in range(2):
                eqk = twide.tile([128, N], F32, tag="eqg")
                rtg = small.tile([128, 1], F32, tag="rtg")
                nc.vector.tensor_scalar_add(rtg[:], io128[:], float(128 * c2))
                kv = small.tile([128, 1], F32, tag="kv")
                nc.vector.tensor_scalar(
                    eqk[:, :NS], mkb[:], rtg[:], None, op0=ALU.is_equal)
                nc.vector.scalar_tensor_tensor(
                    eqk[:, :NS], eqk[:, :NS], 1.0, stb,
                    op0=ALU.bypass, op1=ALU.mult, accum_out=kv[:])
                seg = krow[:, NW + 128 * c2:NW + 128 * (c2 + 1)]
                ldq.dma_start(seg.rearrange("a (k o) -> a k o", o=1), kv[:])

            grow = rows.tile([1, N], F32, tag="grow")
            for c8 in range(8):
                eqg = twide.tile([128, N], F32, tag="eqg")
                ttg = small.tile([128, 1], F32, tag="ttg")
                nc.vector.tensor_scalar_add(ttg[:], io128[:], float(128 * c8))
                gv = small.tile([128, 1], F32, tag="gv")
                nc.vector.tensor_scalar(
                    eqg[:], tkb[:], ttg[:], None, op0=ALU.is_equal)
                nc.vector.scalar_tensor_tensor(
                    eqg[:], eqg[:], 1.0, vlb[:], op0=ALU.bypass, op1=ALU.mult,
                    accum_out=gv[:])
                seg = grow[:, 128 * c8:128 * (c8 + 1)]
                ldq.dma_start(seg.rearrange("a (k o) -> a k o", o=1), gv[:])
            yield

            # int16 gather-idx staging + f32 g row to DRAM
            ki16 = rows.tile([1, T], I16, tag="ki16")
            nc.vector.tensor_copy(ki16[:], krow[:])
            ldq.dma_start(kidx_d[b][None, :], ki16[:])
            for g8 in range(8):
                ldq.dma_start(kidx_t[b][16 * g8:16 * (g8 + 1), :], bass.AP(
                    tensor=kidx_d[b], offset=0, ap=[[1, 16], [16, 32]]))
            ldq.dma_start(g32_d[b][None, :], grow[:])
            if dbg:
                ki32 = rows.tile([1, T], I32, tag="gi32")
                nc.vector.tensor_copy(ki32[:], krow[:])
                nc.sync.dma_start(keep_d[b][None, :], ki32[:])
                gi32 = rows.tile([1, N], I32, tag="gi32b")
                nc.vector.tensor_copy(gi32[:], grow[:])
                nc.sync.dma_start(g_d[b][None, :], gi32[:])

    # tail(b0) fully; then tail(b1) parts woven between frontend(b0) stages
    for _ in index_tail_gen(0):
        pass
    if stop_after == "index":
        for _ in index_tail_gen(1):
            pass
    else:
        f0 = block_frontend(nc, tc, 0, fenv)
        t1 = index_tail_gen(1)
        for g in (t1, f0, t1, f0, t1, f0, t1, f0, t1):
            next(g, None)
        for _ in f0:
            pass
        for _ in t1:
            pass
        for _ in block_frontend(nc, tc, 1, fenv):
            pass

    wA_scope.close()
    tpools.close()

    if stop_after == "index":
        ctx.close()
        return dict(nc=nc)

    # =================== LN2 + MLP + recover ===================
    block_backend(nc, tc, ctx, dict(
        identb=identb, io128=io128, x1=x1, bsm=bsm, out=out,
        wfc1_t=wfc1_t, wfc2_t=wfc2_t, bf1t=bf1t, bfc2_row=bfc2_row,
        g32_d=g32_d))

    ctx.close()
    return dict(nc=nc)


def layer_norm_scalar(nc, pool, small, xin, yout, out_scale=1.0, eps=1e-6):
    """Row LN on the scalar engine (3 passes); DVE only for tiny [128,1] ops.

    yout = (x - mu) * rsqrt(var + eps) * out_scale
    """
    mu = small.tile([128, 1], F32, tag="ln_mu")
    cp = pool.tile([128, D], BF16, tag="lns_cp")
    nc.scalar.activation(cp[:], xin, AF.Copy, accum_out=mu[:])
    nmu = small.tile([128, 1], F32, tag="ln_nmu")
    nc.vector.tensor_scalar_mul(nmu[:], mu[:], -1.0 / D)
    xc = pool.tile([128, D], BF16, tag="lns_xc")
    nc.scalar.activation(xc[:], xin, AF.Identity, bias=nmu[:])
    sq = pool.tile([128, D], BF16, tag="lns_sq")
    var = small.tile([128, 1], F32, tag="ln_var")
    nc.scalar.activation(sq[:], xc[:], AF.Square, accum_out=var[:])
    nc.vector.tensor_scalar(
        var[:], var[:], 1.0 / D, eps, op0=ALU.mult, op1=ALU.add)
    sd = small.tile([128, 1], F32, tag="ln_sd")
    nc.scalar.activation(sd[:], var[:], AF.Sqrt)
    rstd = small.tile([128, 1], F32, tag="ln_rstd")
    nc.vector.reciprocal(rstd[:], sd[:])
    if out_scale != 1.0:
        nc.vector.tensor_scalar_mul(rstd[:], rstd[:], out_scale)
    nc.scalar.activation(yout, xc[:], AF.Copy, scale=rstd[:])


def layer_norm(nc, pool, small, xin, yout, eps=1e-6):
    """Row LN on DVE: yout = (x - mu) * rsqrt(var + eps)."""
    mu = small.tile([128, 1], F32, tag="ln_mu")
    nc.vector.tensor_reduce(mu[:], xin, axis=AX.X, op=ALU.add)
    nc.vector.tensor_scalar_mul(mu[:], mu[:], 1.0 / D)
    xc = pool.tile([128, D], F32, tag="ln_xc")
    nc.vector.tensor_scalar(xc[:], xin, mu[:], None, op0=ALU.subtract)
    sq = pool.tile([128, D], BF16, tag="ln_sq")
    var = small.tile([128, 1], F32, tag="ln_var")
    nc.vector.scalar_tensor_tensor(
        sq[:], xc[:], 1.0, xc[:], op0=ALU.bypass, op1=ALU.mult, accum_out=var[:])
    nc.vector.tensor_scalar(
        var[:], var[:], 1.0 / D, eps, op0=ALU.mult, op1=ALU.add)
    sd = small.tile([128, 1], F32, tag="ln_sd")
    nc.scalar.activation(sd[:], var[:], AF.Sqrt)
    rstd = small.tile([128, 1], F32, tag="ln_rstd")
    nc.vector.reciprocal(rstd[:], sd[:])
    nc.vector.tensor_scalar_mul(yout, xc[:], rstd[:])


def block_frontend(nc, tc, b, E):
    """LN1 -> V -> QK -> attention -> proj for one sample."""
    identb, io128 = E["identb"], E["io128"]
    x_in, x1, bsm = E["x_in"], E["x1"], E["bsm"]
    wqkA, wvA, wpA = E["wqkA"], E["wvA"], E["wpA"]

    with (
        tc.tile_pool(name="p_sb", bufs=1) as p_sb,
    ):
        YT = p_sb.tile([128, DC, T], FP8, tag="YT")
        Vaug = p_sb.tile([128, TC, H, 97], FP8, tag="Vaug")
        OT = p_sb.tile([128, DC, T], FP8, tag="OT")
        QKT = p_sb.tile([128, 18, T], FP8, tag="QKT")
        nc.vector.memset(Vaug[:, :, :, DH:96], 0.0)
        nc.vector.memset(Vaug[:, :, :, 96:97], 1.0)

        # ---- LN1 -> YT (keep-token gather; x1 seeded with residual) ----
        with (
            tc.tile_pool(name="p_xk", bufs=1) as p_xk,
            tc.tile_pool(name="p_ln1", bufs=2) as p_ln,
            tc.tile_pool(name="psT1", bufs=2, space="PSUM") as psT,
        ):
            xk = p_xk.tile([128, TC, D], F32, name=f"xk{b}")
            for c4 in range(TC):
                nc.gpsimd.dma_gather(
                    out_ap=xk[:, c4:c4 + 1, :], in_ap=x_in[b],
                    idxs_ap=E["kidx_t"][b][:, 8 * c4:8 * (c4 + 1)],
                    num_idxs=128, num_idxs_reg=128, elem_size=D)
                y = p_ln.tile([128, D], BF16, tag="y")
                layer_norm_scalar(nc, p_ln, bsm, xk[:, c4, :], y[:],
                                  out_scale=S_Y)
                for dc in range(DC):
                    pt = psT.tile([128, 128], BF16, tag="bt")
                    nc.tensor.transpose(
                        pt[:], y[:, 128 * dc:128 * (dc + 1)], identb[:])
                    nc.scalar.copy(YT[:, dc, 128 * c4:128 * (c4 + 1)], pt[:])
                nc.vector.tensor_add(x1[:, b, c4, :], xk[:, c4, :], E["bpb"][:])
        yield

        # ---- V (fp8 DoubleRow over D-chunk pairs) ----
        with tc.tile_pool(name="psV", bufs=3, space="PSUM") as psV:
            for c4 in range(TC):
                for g4 in range(4):
                    pv = psV.tile([128, 4 * DH], F32, tag="pv")
                    for i in range(4):
                        nc.tensor.matmul(
                            pv[:],
                            YT[:, 2 * i:2 * i + 2, 128 * c4:128 * (c4 + 1)],
                            wvA[:, 2 * i:2 * i + 2,
                                4 * DH * g4:4 * DH * (g4 + 1)],
                            start=(i == 0), stop=False, perf_mode=DR)
                    nc.tensor.matmul(
                        pv[:], YT[:, 8, 128 * c4:128 * (c4 + 1)],
                        wvA[:, 8, 4 * DH * g4:4 * DH * (g4 + 1)],
                        start=False, stop=True)
                    nc.vector.scalar_tensor_tensor(
                        Vaug[:, c4, 4 * g4:4 * (g4 + 1), 0:DH],
                        pv[:].rearrange("p (h d) -> p h d", h=4), V_EVAC,
                        E["bvb"][:, 4 * DH * g4:4 * DH * (g4 + 1)]
                        .rearrange("p (h d) -> p h d", h=4),
                        op0=ALU.mult, op1=ALU.add)
        yield

        # ---- Q/K for all heads (fp8 DoubleRow; evac on scalar engine) ----
        with tc.tile_pool(name="psQ", bufs=3, space="PSUM") as psQ:
            for mc in range(18):
                pq = psQ.tile([128, T], F32, tag="pq")
                for i in range(4):
                    nc.tensor.matmul(
                        pq[:], wqkA[:, mc, 2 * i:2 * i + 2, :],
                        YT[:, 2 * i:2 * i + 2, :],
                        start=(i == 0), stop=False, perf_mode=DR)
                nc.tensor.matmul(
                    pq[:], wqkA[:, mc, 8, :], YT[:, 8, :],
                    start=False, stop=True)
                nc.scalar.activation(
                    QKT[:, mc, :], pq[:], AF.Identity,
                    scale=QK_EVAC, bias=E["bqkt"][:, mc:mc + 1])
        yield

        # ---- attention ----
        with (
            tc.tile_pool(name="p_att", bufs=2) as p_att,
            tc.tile_pool(name="p_qkh", bufs=2) as p_qkh,
            tc.tile_pool(name="p_et", bufs=3) as p_et,
            tc.tile_pool(name="p_po", bufs=2) as p_po,
            tc.tile_pool(name="p_rs", bufs=2) as p_rs,
            tc.tile_pool(name="psS", bufs=2, space="PSUM") as psS,
            tc.tile_pool(name="psO", bufs=2, space="PSUM") as psO,
            tc.tile_pool(name="psC", bufs=2, space="PSUM") as psC,
        ):
            for hg in range(2):
                qh8 = p_qkh.tile([DH, 8, T], FP8, tag="qh8")
                kh8 = p_qkh.tile([DH, 8, T], FP8, tag="kh8")
                for (dst8, base) in ((qh8, 0), (kh8, 576)):
                    r0 = base
                    while r0 < base + 8 * DH:
                        mcl, p0 = divmod(r0, 128)
                        hl, d0 = divmod(r0 - base, DH)
                        take = min(128 - p0, DH - d0)
                        nc.gpsimd.dma_start(
                            dst8[d0:d0 + take, hl, :],
                            QKT[p0:p0 + take, DC * hg + mcl, :])
                        r0 += take
                posb = p_po.tile([128, 8, T], BF16, tag="posb")
                for hl in range(8):
                    h = 8 * hg + hl
                    ET = p_et.tile([128, TC, T], FP8, tag="ET")
                    for half in range(2):
                        st2 = psS.tile([128, 2, T], F32, tag="st2")
                        for kcl in range(2):
                            kc = 2 * half + kcl
                            nc.tensor.matmul(
                                st2[:, kcl, :],
                                kh8[:, hl, 128 * kc:128 * (kc + 1)],
                                qh8[:, hl, :],
                                start=True, stop=True)
                        nc.scalar.activation(
                            ET[:, 2 * half:2 * (half + 1), :], st2[:],
                            AF.Exp, scale=EXP_SCALE, bias=E["expb"][:])
                    po = psO.tile([128, T], F32, tag="po")
                    for g in range(2):
                        nc.tensor.matmul(
                            po[:97, :],
                            Vaug[:, 2 * g:2 * g + 2, h, :],
                            ET[:, 2 * g:2 * g + 2, :],
                            start=(g == 0), stop=(g == 1), perf_mode=DR)
                    nc.scalar.copy(posb[:97, hl, :], po[:97, :])
                rs8 = p_po.tile([8, T], BF16, tag="rs8")
                for hl in range(8):
                    nc.sync.dma_start(
                        rs8[hl:hl + 1, :], posb[96:97, hl, :])
                rs8i = p_po.tile([8, T], BF16, tag="rs8i")
                with nc.allow_low_precision(
                        reason="softmax 1/rsum in bf16 matches block dtype"):
                    nc.vector.reciprocal(rs8i[:], rs8[:])
                for hl in range(8):
                    h = 8 * hg + hl
                    rsh = p_rs.tile([1, T], BF16, tag="rsh")
                    nc.sync.dma_start(rsh[:], rs8i[hl:hl + 1, :])
                    bc = psC.tile([128, T], F32, tag="bc")
                    nc.tensor.matmul(
                        bc[:DH, :], E["ones72"][:], rsh[:],
                        start=True, stop=True)
                    oh = p_att.tile([DH, T], FP8, tag="oh")
                    nc.vector.tensor_mul(
                        oh[:], posb[:DH, hl, :], bc[:DH, :])
                    r0 = DH * h
                    while r0 < DH * (h + 1):
                        dc, p0 = divmod(r0, 128)
                        take = min(128 - p0, DH * (h + 1) - r0)
                        nc.sync.dma_start(
                            OT[p0:p0 + take, dc, :],
                            oh[r0 - DH * h:r0 - DH * h + take, :])
                        r0 += take
                if hg == 0:
                    yield

        # ---- proj (fp8 DoubleRow; bf16 residual accumulate in place) ----
        with tc.tile_pool(name="psP", bufs=3, space="PSUM") as psP:
            for c4 in range(TC):
                for ns in range(3):
                    pp = psP.tile([128, 384], F32, tag="pp")
                    for i in range(4):
                        nc.tensor.matmul(
                            pp[:],
                            OT[:, 2 * i:2 * i + 2, 128 * c4:128 * (c4 + 1)],
                            wpA[:, 2 * i:2 * i + 2, 384 * ns:384 * (ns + 1)],
                            start=(i == 0), stop=False, perf_mode=DR)
                    nc.tensor.matmul(
                        pp[:], OT[:, 8, 128 * c4:128 * (c4 + 1)],
                        wpA[:, 8, 384 * ns:384 * (ns + 1)],
                        start=False, stop=True)
                    sl = x1[:, b, c4, 384 * ns:384 * (ns + 1)]
                    nc.vector.scalar_tensor_tensor(
                        sl, pp[:], PROJ_EVAC, sl, op0=ALU.mult, op1=ALU.add)


def block_backend(nc, tc, ctx, E):
    """LN2 + MLP (three 12-chunk thirds) + recover, both samples."""
    identb, io128, x1, bsm, out = (
        E["identb"], E["io128"], E["x1"], E["bsm"], E["out"])
    wfc1_t, wfc2_t, bf1t = E["wfc1_t"], E["wfc2_t"], E["bf1t"]
    g32_d = E["g32_d"]

    recp = ctx.enter_context(tc.tile_pool(name="recp", bufs=1))
    psR = ctx.enter_context(tc.tile_pool(name="psR", bufs=2, space="PSUM"))

    with tc.tile_pool(name="p_y2", bufs=1) as p_y2:
        Y2T = p_y2.tile([128, DC, 2 * T], BF16)
        bf2b = p_y2.tile([128, D], BF16, name="bf2b")
        nc.gpsimd.dma_start(bf2b[:], bass.AP(
            tensor=E["bfc2_row"], offset=0, ap=[[0, 128], [1, D]]))
        with (
            tc.tile_pool(name="p_ln2", bufs=2) as p_ln,
            tc.tile_pool(name="psT2", bufs=2, space="PSUM") as psT,
        ):
            for ct in range(8):
                b, c4 = divmod(ct, TC)
                y = p_ln.tile([128, D], BF16, tag="y")
                layer_norm(nc, p_ln, bsm, x1[:, b, c4, :], y[:])
                for dc in range(DC):
                    pt = psT.tile([128, 128], BF16, tag="bt")
                    nc.tensor.transpose(
                        pt[:], y[:, 128 * dc:128 * (dc + 1)], identb[:])
                    nc.scalar.copy(Y2T[:, dc, 128 * ct:128 * (ct + 1)], pt[:])
                nc.vector.tensor_add(
                    x1[:, b, c4, :], x1[:, b, c4, :], bf2b[:])

        with (
            tc.tile_pool(name="p_ht", bufs=2) as p_ht,
            tc.tile_pool(name="p_wf1", bufs=4) as p_wf1,
            tc.tile_pool(name="p_wf2", bufs=2) as p_wf2,
            tc.tile_pool(name="psA2", bufs=3, space="PSUM") as psA,
            tc.tile_pool(name="psB2", bufs=3, space="PSUM") as psB,
        ):
            for g in range(3):
                wf2t = p_wf2.tile([128, 12, D], BF16, tag="wf2t")
                nc.gpsimd.dma_start(
                    wf2t[:], wfc2_t[:, 12 * g:12 * (g + 1), :])
                HT = p_ht.tile([128, 12, 2 * T], BF16, tag="HT")
                for k12 in range(12):
                    mf = 12 * g + k12
                    wt = p_wf1.tile([128, DC, 128], BF16, tag="wf1")
                    nc.scalar.dma_start(wt[:], wfc1_t[mf])
                    for nh in range(2):
                        pf = psA.tile([128, T], F32, tag="a")
                        for dc in range(DC):
                            nc.tensor.matmul(
                                pf[:], wt[:, dc, :],
                                Y2T[:, dc, T * nh:T * (nh + 1)],
                                start=(dc == 0), stop=(dc == DC - 1))
                        nc.scalar.activation(
                            HT[:, k12, T * nh:T * (nh + 1)], pf[:],
                            AF.Gelu_apprx_tanh, bias=bf1t[:, mf:mf + 1])
                last = (g == 2)
                for b in range(B2):
                    for c4 in range(TC):
                        ct = TC * b + c4
                        for ns in range(3):
                            pg = psB.tile([128, 384], F32, tag="b")
                            for i in range(12):
                                nc.tensor.matmul(
                                    pg[:], HT[:, i, 128 * ct:128 * (ct + 1)],
                                    wf2t[:, i, 384 * ns:384 * (ns + 1)],
                                    start=(i == 0), stop=(i == 11))
                            sl = x1[:, b, c4, 384 * ns:384 * (ns + 1)]
                            nc.vector.scalar_tensor_tensor(
                                sl, pg[:], 1.0, sl,
                                op0=ALU.bypass, op1=ALU.add)
                    if last:
                        recover_sample(nc, recp, psR, io128, g32_d, x1, out, b)


def recover_sample(nc, recp, psR, io128, g32_d, x1, out, b):
    """out[b, t] = x1[:, b, g[t]] via one-hot permutation matmuls."""
    gb = recp.tile([128, N], F32, tag="gb")
    nc.sync.dma_start(gb[:], bass.AP(
        tensor=g32_d[b], offset=0, ap=[[0, 128], [1, N]]))
    pc4 = recp.tile([128, TC], F32, tag="pc4")
    for c4 in range(TC):
        nc.vector.tensor_scalar_add(
            pc4[:, c4:c4 + 1], io128[:], float(128 * c4))
    for c8 in range(8):
        Mt = recp.tile([128, TC, 128], BF16, tag="Mt")
        for c4 in range(TC):
            nc.vector.tensor_scalar(
                Mt[:, c4, :], gb[:, 128 * c8:128 * (c8 + 1)],
                pc4[:, c4:c4 + 1], None, op0=ALU.is_equal)
        og = recp.tile([128, D], F32, tag="og")
        for ns in range(3):
            pr = psR.tile([128, 384], F32, tag="pr")
            for c4 in range(TC):
                nc.tensor.matmul(
                    pr[:], Mt[:, c4, :],
                    x1[:, b, c4, 384 * ns:384 * (ns + 1)],
                    start=(c4 == 0), stop=(c4 == TC - 1))
            nc.scalar.copy(og[:, 384 * ns:384 * (ns + 1)], pr[:])
        nc.sync.dma_start(
            bass.AP(tensor=out, offset=(b * N + 128 * c8) * D,
                    ap=[[D, 128], [1, D]]),
            og[:])


# ======================================================================
# kernel() entry point: full inputs -> full output on 8 NeuronCores
# ======================================================================

_MODULE_CACHE = {}


def _get_module(key="fp8attn"):
    if key not in _MODULE_CACHE:
        from concourse import bacc
        nc = bacc.Bacc(None, target_bir_lowering=False)
        build(nc, {})
        nc.compile()
        _MODULE_CACHE[key] = nc
    return _MODULE_CACHE[key]


def kernel(x, noise, ln1_g, ln1_b, ln2_g, ln2_b, w_qkv, b_qkv, w_proj, b_proj,
           w_fc1, b_fc1, w_fc2, b_fc2, block_dtype="bf16", **run_kw):
    from concourse import bass_utils

    x = np.ascontiguousarray(np.asarray(x, np.float32))
    noise = np.ascontiguousarray(np.asarray(noise, np.float32))
    B = x.shape[0]
    n_cores = B // B2
    wt = retile_weights(
        dict(ln1_g=ln1_g, ln1_b=ln1_b, ln2_g=ln2_g, ln2_b=ln2_b,
             w_qkv=w_qkv, b_qkv=b_qkv, w_proj=w_proj, b_proj=b_proj,
             w_fc1=w_fc1, b_fc1=b_fc1, w_fc2=w_fc2, b_fc2=b_fc2))

    nc = _get_module()
    in_maps = []
    for c in range(n_cores):
        m = dict(x=x[B2 * c:B2 * (c + 1)], noise=noise[B2 * c:B2 * (c + 1)])
        m.update(wt)
        in_maps.append(m)
    res = bass_utils.run_bass_kernel_spmd(
        nc, in_maps, core_ids=list(range(n_cores)), **run_kw)
    out = np.concatenate([res.results[c]["out"] for c in range(n_cores)], axis=0)
    if run_kw.get("trace"):
        return out, res
    return out
